# revision 4
# baseline (speedup 1.0000x reference)
"""Bass/Trainium2 kernel for nn_BayesMultiheadAttention (B=4,T=2048,D=1024,H=8).

Sharding: tensor-parallel over heads. Core c computes head c (QKV proj +
causal attention) for all 4 batches; a per-batch fp16 AllToAll
redistributes per-head outputs into per-token-slice outputs (consumed two
batches later, so the collective is never on the critical path); each
core then does the multiplicative reduce over heads and its slice of
out_proj.

v2 changes vs v1:
- Attention operands (qT/kT/vT/v_sb/e/es/cm/ones/eye) are fp16: PE
  streams 1 elem/cycle at any N for 16-bit moving operands (f32r drops
  to 1/4 rate below N=256), transposes run 1 cyc/row, and DVE gets 2x
  throughput on the elementwise chain.
- Scores matmuls for the 4 diagonal-crossing tiles of each q-chunk only
  compute the valid column suffix (N = 512-128j), and exp reads the same
  suffix; the causal cm-mask multiply covers the complementary prefix
  [0,(j+1)*128) so stale columns are zeroed. Saves ~20% of score/exp
  work.
- exp runs once per PAIR of full score tiles (ACT reads a 2-bank
  [128,1024] PSUM span): ACT's per-instruction overhead is 352 cycles,
  so halving the instruction count saves ~3.5us/batch; ACT was
  co-critical with PE at 40 exps/batch.
- Softmax denominator: e-tiles are summed in QUADS on DVE (pair adds
  from each e-pair tile, then one cross-pair add) so PE runs one
  ones-matmul per 4 k-tiles (10/batch instead of 20).
- The second-to-last batch's tail is consumed during the last batch's
  attention (qc==3) instead of after the loop, hiding ~10us of the
  serial tail.

Queues: PE matmuls only; ACT exp (+ startup loads); DVE masks/recip/
normalize + es-adds; Pool collectives, a2a staging, tail gather+product
chain; SP x/w loads and y stores. PSUM evictions use nc.any so the tile
scheduler balances engines. build_program(reps=k) chains the whole
pipeline k times in one NEFF for launch-overhead-free timing.
"""
import numpy as np

import concourse.bacc as bacc
import concourse.mybir as mybir
import concourse.tile as tile
from concourse.bass_utils import run_bass_kernel_spmd

B, T, D, H = 4, 2048, 1024, 8
HD = 128          # head dim
P = 128           # partitions
NC = 8            # cores
TQ = 512          # qt chunk width
NKD = D // P      # 8 contraction tiles
NTT = T // P      # 16 token tiles per batch
NQC = T // TQ     # 4 qt chunks per batch
TS = T // NC      # 256: per-core token slice of one batch
TOK_SLICE = B * TS  # 1024 tokens per core in the tail

dt = mybir.dt
F32 = dt.float32
F32R = dt.float32r
F16 = dt.float16

_PROGRAM = {}


def build_program(mode=None, reps=1):
    """reps>1 repeats the whole pipeline inside one NEFF (for timing)."""
    global _PROGRAM
    key = f"v3-{reps}"
    if key in _PROGRAM:
        return _PROGRAM[key]
    nc = bacc.Bacc("TRN2", target_bir_lowering=False, debug=False,
                   num_devices=NC)

    xT_d = nc.dram_tensor("xT", [B, D, T], F16, kind="ExternalInput")
    wq_d = nc.dram_tensor("wq", [B, P, NKD * HD], F16, kind="ExternalInput")
    wk_d = nc.dram_tensor("wk", [B, P, NKD * HD], F16, kind="ExternalInput")
    wv_d = nc.dram_tensor("wv", [B, P, NKD * HD], F16, kind="ExternalInput")
    wo_d = nc.dram_tensor("wo", [HD, D], F32R, kind="ExternalInput")
    cm_d = nc.dram_tensor("cm", [4, P, TQ], F16, kind="ExternalInput")
    eye_d = nc.dram_tensor("eye", [P, P], F16, kind="ExternalInput")
    y_d = nc.dram_tensor("y", [TOK_SLICE, D], F32, kind="ExternalOutput")

    rg = [list(range(NC))]
    Exp = mybir.ActivationFunctionType.Exp

    from contextlib import ExitStack
    with tile.TileContext(nc) as tc, ExitStack() as ctx:
        ec = ctx.enter_context
        constp = ec(tc.tile_pool(name="const", bufs=1))
        xp = ec(tc.tile_pool(name="xp", bufs=2))
        wsp = ec(tc.tile_pool(name="wsp", bufs=2))
        qkvp = ec(tc.tile_pool(name="qkv", bufs=1))
        eop = ec(tc.tile_pool(name="eo", bufs=4))
        esp = ec(tc.tile_pool(name="es", bufs=4))
        scp = ec(tc.tile_pool(name="sc", bufs=3))
        outbp = ec(tc.tile_pool(name="outb", bufs=1))
        prodp = ec(tc.tile_pool(name="prod", bufs=2))
        ysbp = ec(tc.tile_pool(name="ysb", bufs=2))
        # PSUM: psS holds 2-bank [P,1024] score-pair tiles (exp reads the
        # span in one instruction); psO/psD are 1-bank acco/denb rings.
        psS = ec(tc.tile_pool(name="psS", bufs=2, space="PSUM"))
        psO = ec(tc.tile_pool(name="psO", bufs=2, space="PSUM"))
        psD = ec(tc.tile_pool(name="psD", bufs=2, space="PSUM"))
        dram = ec(tc.tile_pool(name="dram", bufs=1, space="DRAM"))

        a2a_in = [dram.tile([NC, P, TS], F16, name=f"a2a_in{b}",
                            tag=f"a2a_in{b}") for b in range(B)]
        a2a_out = [dram.tile([NC, P, TS], F16, name=f"a2a_out{b}",
                             tag=f"a2a_out{b}") for b in range(B)]
        hpp = ec(tc.tile_pool(name="hp", bufs=2))

        ones_h = constp.tile([P, P], F16, name="ones_h", tag="ones_h")
        nc.vector.memset(ones_h[:], 1.0)

        eye_h = constp.tile([P, P], F16, name="eye_h", tag="eye_h")
        nc.sync.dma_start(eye_h[:], eye_d.ap())

        cm_sb = constp.tile([P, 4 * TQ], F16, name="cm_sb", tag="cm_sb")

        wor = constp.tile([P, D], F32R, name="wor", tag="wor")

        # ---- load machinery -------------------------------------------
        staged = {}

        def emit_loads(bb, startup=False):
            """DMA batch bb's x chunks + weights; record tiles in staged."""
            b = bb % B
            st = {}
            staged[bb] = st
            x_sb = xp.tile([P, NKD * T], F16, name="x_sb", tag="x_sb")
            st["x"] = x_sb
            ws = {}
            for nm, wd in (("v", wv_d), ("q", wq_d), ("k", wk_d)):
                ws[nm] = wsp.tile([P, NKD * HD], F16, name=f"ws_{nm}",
                                  tag=f"ws_{nm}")
            st["w"] = ws
            if startup:
                # spread batch-0 loads over Pool/SP/ACT in half-chunk DMAs
                # so the kd-outer v projection starts ~2us in and is never
                # starved; wv first on ACT
                nc.scalar.dma_start(ws["v"][:], wv_d.ap()[b])
                engs = [nc.gpsimd, nc.sync, nc.scalar]
                i = 0
                HT = T // 2
                QT4 = T // 4
                for kd in range(NKD):
                    if kd == 0:
                        for h in range(4):
                            engs[i % 3].dma_start(
                                x_sb[:, h * QT4:(h + 1) * QT4],
                                xT_d.ap()[b, 0:P, h * QT4:(h + 1) * QT4])
                            i += 1
                        continue
                    for h in range(2):
                        engs[i % 3].dma_start(
                            x_sb[:, kd * T + h * HT: kd * T + (h + 1) * HT],
                            xT_d.ap()[b, kd * P:(kd + 1) * P,
                                      h * HT:(h + 1) * HT])
                        i += 1
                    if kd == 2:
                        nc.sync.dma_start(ws["q"][:], wq_d.ap()[b])
                    elif kd == 4:
                        nc.gpsimd.dma_start(ws["k"][:], wk_d.ap()[b])
                return
            # steady state: all on SP; few big DMAs (HW A/B showed the
            # consolidated transfers beat per-chunk DMAs by ~90us)
            HK = NKD // 2
            nc.sync.dma_start(ws["v"][:], wv_d.ap()[b])
            nc.sync.dma_start(
                x_sb[:, 0:HK * T],
                xT_d.ap()[b, 0:HK * P, :].rearrange("(kd p) t -> p kd t",
                                                    p=P))
            nc.sync.dma_start(ws["q"][:], wq_d.ap()[b])
            nc.sync.dma_start(
                x_sb[:, HK * T:NKD * T],
                xT_d.ap()[b, HK * P:NKD * P, :].rearrange(
                    "(kd p) t -> p kd t", p=P))
            nc.sync.dma_start(ws["k"][:], wk_d.ap()[b])

        def emit_tail(bb, final=False):
            """Consume A2A(bb): f16 head-product chain (Pool only), out_proj."""
            b = bb % B
            hp = hpp.tile([P, NC * TS], F16, name="hp", tag="hp")
            if final:
                half = NC // 2
                nc.gpsimd.dma_start(
                    hp[:, 0:half * TS],
                    a2a_out[b][0:half].rearrange("r p t -> p r t"))
                nc.scalar.dma_start(
                    hp[:, half * TS:],
                    a2a_out[b][half:NC].rearrange("r p t -> p r t"))
            else:
                nc.gpsimd.dma_start(
                    hp[:], a2a_out[b].rearrange("r p t -> p r t"))
            pr = prodp.tile([P, TS], F16, name="pr", tag="pr")
            nc.gpsimd.tensor_mul(pr[:], hp[:, 0:TS], hp[:, TS:2 * TS])
            for r in range(2, NC - 1):
                nc.gpsimd.tensor_mul(
                    pr[:], pr[:], hp[:, r * TS:(r + 1) * TS])
            prod_r = prodp.tile([P, TS], F32R, name="prod_r", tag="prodr")
            nc.gpsimd.tensor_mul(
                prod_r[:], pr[:], hp[:, (NC - 1) * TS:NC * TS])
            for tt in range(TS // P):
                ysb = ysbp.tile([P, D], F32, name="ysb", tag="ysb")
                for nn in range(D // TQ):
                    accy = psO.tile([P, TQ], F32, name="accy", tag="acco")
                    nc.tensor.matmul(
                        accy[:],
                        prod_r[:, tt * P:(tt + 1) * P],
                        wor[:, nn * TQ:(nn + 1) * TQ],
                        start=True, stop=True)
                    nc.vector.tensor_copy(
                        ysb[:, nn * TQ:(nn + 1) * TQ], accy[:])
                row = b * TS + tt * P
                yeng = nc.scalar if (final and tt == 1) else nc.sync
                yeng.dma_start(y_d.ap()[row:row + P, :], ysb[:])

        emit_loads(0, startup=True)
        nc.scalar.dma_start(cm_sb[:], cm_d.ap().rearrange("j p q -> p j q"))
        nc.gpsimd.dma_start(wor[:], wo_d.ap())

        NB = B * reps
        for bb in range(NB):
            b = bb % B
            st = staged.pop(bb)
            if bb + 1 < NB:
                emit_loads(bb + 1)
            xr = st["x"]
            wvr = st["w"]["v"]
            wqr = st["w"]["q"]
            wkr = st["w"]["k"]

            # ---- V projection, kd-outer (4 chunk accumulators) --------
            # accumulators: psS tile halves (2 banks) + 2 psD tiles
            vT = qkvp.tile([P, T], F16, name="vT", tag="vT")
            sv = psS.tile([P, 2 * TQ], F32, name="sv", tag="accs")
            dv = [psD.tile([P, TQ], F32, name=f"dv{i}", tag="denb")
                  for i in range(2)]
            vaccs = [sv[:, 0:TQ], sv[:, TQ:2 * TQ], dv[0][:], dv[1][:]]
            for kd in range(NKD):
                for qc in range(NQC):
                    nc.tensor.matmul(
                        vaccs[qc],
                        wvr[:, kd * HD:(kd + 1) * HD],
                        xr[:, kd * T + qc * TQ: kd * T + (qc + 1) * TQ],
                        start=(kd == 0), stop=(kd == NKD - 1))
            for qc in range(NQC):
                nc.any.tensor_copy(
                    vT[:, qc * TQ:(qc + 1) * TQ], vaccs[qc])

            # flip V to (token partitions, hd free) via fp16 PE transposes
            v_sb = qkvp.tile([P, NTT * HD], F16, name="vS", tag="vS")
            for tt in range(NTT):
                vtp = psO.tile([P, P], F16, name="vtp", tag="acco")
                nc.tensor.transpose(
                    vtp[:], vT[:, tt * P:(tt + 1) * P], eye_h[:])
                nc.any.tensor_copy(
                    v_sb[:, tt * HD:(tt + 1) * HD], vtp[:])

            # ---- Q,K projections, kd-outer (8 chunk accumulators) -----
            qT = qkvp.tile([P, T], F16, name="qT", tag="qT")
            kT = qkvp.tile([P, T], F16, name="kT", tag="kT")
            sq = psS.tile([P, 2 * TQ], F32, name="sq", tag="accs")
            oq = [psO.tile([P, TQ], F32, name=f"oq{i}", tag="acco")
                  for i in range(2)]
            qaccs = [sq[:, 0:TQ], sq[:, TQ:2 * TQ], oq[0][:], oq[1][:]]
            sk = psS.tile([P, 2 * TQ], F32, name="sk", tag="accs")
            dk = [psD.tile([P, TQ], F32, name=f"dk{i}", tag="denb")
                  for i in range(2)]
            kaccs = [sk[:, 0:TQ], sk[:, TQ:2 * TQ], dk[0][:], dk[1][:]]
            for kd in range(NKD):
                for qc in range(NQC):
                    nc.tensor.matmul(
                        qaccs[qc],
                        wqr[:, kd * HD:(kd + 1) * HD],
                        xr[:, kd * T + qc * TQ: kd * T + (qc + 1) * TQ],
                        start=(kd == 0), stop=(kd == NKD - 1))
                for qc in range(NQC):
                    nc.tensor.matmul(
                        kaccs[qc],
                        wkr[:, kd * HD:(kd + 1) * HD],
                        xr[:, kd * T + qc * TQ: kd * T + (qc + 1) * TQ],
                        start=(kd == 0), stop=(kd == NKD - 1))
            for qc in range(NQC):
                nc.any.tensor_copy(
                    qT[:, qc * TQ:(qc + 1) * TQ], qaccs[qc])
                nc.any.tensor_copy(
                    kT[:, qc * TQ:(qc + 1) * TQ], kaccs[qc])

            # ---- causal attention, scoresT layout, pair-granular ------
            out_b = outbp.tile([P, T], F16, name="out_b", tag="out_b")
            for qc in range(NQC):
                nkt = 4 * (qc + 1)
                nfull = 4 * qc
                acco = psO.tile([P, TQ], F32, name="acco", tag="acco")
                denb = psD.tile([P, TQ], F32, name="denb", tag="denb")
                es_pairs = []

                for pr_i in range(nkt // 2):
                    kt0 = 2 * pr_i
                    sp = psS.tile([P, 2 * TQ], F32, name="sp", tag="accs")
                    ep = eop.tile([P, 2 * TQ], F16, name="ep", tag="ep")
                    for h in (0, 1):
                        kt = kt0 + h
                        j = kt - nfull
                        off = 0 if j < 0 else j * P  # valid-column suffix
                        nc.tensor.matmul(
                            sp[:, h * TQ + off:(h + 1) * TQ],
                            kT[:, kt * P:(kt + 1) * P],
                            qT[:, qc * TQ + off:(qc + 1) * TQ],
                            start=True, stop=True)
                    j0 = kt0 - nfull
                    if j0 < 0:
                        # both tiles full: one exp over the 2-bank span
                        nc.scalar.activation(ep[:], sp[:], Exp)
                    else:
                        for h in (0, 1):
                            j = j0 + h
                            off = j * P
                            nc.scalar.activation(
                                ep[:, h * TQ + off:(h + 1) * TQ],
                                sp[:, h * TQ + off:(h + 1) * TQ], Exp)
                            # zero the invalid prefix (never written by the
                            # suffix exp; slot memory is stale) and mask the
                            # diagonal-crossing 128-col block
                            if j > 0:
                                nc.vector.memset(ep[:, h * TQ:h * TQ + off],
                                                 0.0)
                            nc.vector.tensor_mul(
                                ep[:, h * TQ + off:h * TQ + off + P],
                                ep[:, h * TQ + off:h * TQ + off + P],
                                cm_sb[:, j * TQ + off:j * TQ + off + P])
                    for h in (0, 1):
                        kt = kt0 + h
                        nc.tensor.matmul(
                            acco[:],
                            v_sb[:, kt * HD:(kt + 1) * HD],
                            ep[:, h * TQ:(h + 1) * TQ],
                            start=(kt == 0), stop=(kt == nkt - 1))
                    es = esp.tile([P, TQ], F16, name="es", tag="es")
                    nc.vector.tensor_add(
                        es[:], ep[:, 0:TQ], ep[:, TQ:2 * TQ])
                    es_pairs.append(es)
                    if len(es_pairs) == 2:
                        esq = esp.tile([P, TQ], F16, name="esq", tag="esq")
                        nc.vector.tensor_add(
                            esq[:], es_pairs[0][:], es_pairs[1][:])
                        es_pairs = []
                        quad = pr_i // 2
                        nc.tensor.matmul(
                            denb[:], ones_h[:], esq[:],
                            start=(quad == 0), stop=(quad == qc))

                recb = scp.tile([P, TQ], F32, name="recb", tag="recb")
                nc.vector.reciprocal_approx_fast(recb[:], denb[:])
                nc.vector.tensor_mul(
                    out_b[:, qc * TQ:(qc + 1) * TQ], acco[:], recb[:])
                # ship this qc's two token slices to the collective buffer
                for j in (2 * qc, 2 * qc + 1):
                    nc.gpsimd.dma_start(a2a_in[b][j],
                                        out_b[:, j * TS:(j + 1) * TS])

                if qc == 1 and bb > 1:
                    emit_tail(bb - 2)
                if qc == 3 and bb == NB - 1 and NB > 1:
                    emit_tail(bb - 1)

            # ---- ship normalized head-output (f16) ---------------------
            nc.gpsimd.collective_compute(
                "AllToAll", mybir.AluOpType.bypass,
                replica_groups=rg,
                ins=[a2a_in[b].opt()], outs=[a2a_out[b].opt()])

        emit_tail(NB - 1, final=True)

    nc.compile()
    _PROGRAM[key] = nc
    return nc


def make_in_maps(x, Wq, Wk, Wv, Wout, q_mask, k_mask, v_mask):
    x = np.ascontiguousarray(np.asarray(x, np.float32))
    xT = np.ascontiguousarray(x.transpose(0, 2, 1).astype(np.float16))
    wo = np.ascontiguousarray(np.asarray(Wout, np.float32).T)  # (HD, D)

    cm = np.zeros((4, P, TQ), np.float16)
    for j in range(4):
        for i in range(P):
            cm[j, i, j * P + i:] = 1.0
    eye = np.eye(P, dtype=np.float16)

    s = np.float32(1.0 / np.sqrt(HD))
    q_mask = np.asarray(q_mask, np.float32)
    k_mask = np.asarray(k_mask, np.float32)
    v_mask = np.asarray(v_mask, np.float32)
    Wq = np.asarray(Wq, np.float32)
    Wk = np.asarray(Wk, np.float32)
    Wv = np.asarray(Wv, np.float32)

    in_maps = []
    for c in range(NC):
        def pack(W, m, scale):
            out = np.empty((B, P, NKD * HD), np.float16)
            Wh = W[c * HD:(c + 1) * HD, :]                  # (HD, D)
            for b in range(B):
                Wp = (Wh * (m[b, c, 0, :, None] * scale)).T  # (D, HD)
                out[b] = Wp.reshape(NKD, P, HD).transpose(1, 0, 2).reshape(
                    P, NKD * HD)
            return out
        in_maps.append({
            "xT": xT,
            "wq": pack(Wq, q_mask, s),
            "wk": pack(Wk, k_mask, np.float32(1.0)),
            "wv": pack(Wv, v_mask, np.float32(1.0)),
            "wo": wo,
            "cm": cm,
            "eye": eye,
        })
    return in_maps


def kernel(x, Wq, Wk, Wv, Wout, q_mask, k_mask, v_mask, mask=None):
    nc = build_program()
    in_maps = make_in_maps(x, Wq, Wk, Wv, Wout, q_mask, k_mask, v_mask)
    res = run_bass_kernel_spmd(nc, in_maps, core_ids=list(range(NC))).results
    # core c's y rows are ordered (b, local-token); its tokens are
    # [c*TS, (c+1)*TS) of every batch
    out = np.empty((B, T, D), np.float32)
    for c in range(NC):
        yc = res[c]["y"].reshape(B, TS, D)
        out[:, c * TS:(c + 1) * TS, :] = yc
    return out


# revision 5
# speedup vs baseline: 1.0336x; 1.0336x over previous
"""Bass/Trainium2 kernel for nn_BayesMultiheadAttention (B=4,T=2048,D=1024,H=8).

Sharding: tensor-parallel over heads. Core c computes head c (QKV proj +
causal attention) for all 4 batches; a per-batch fp16 AllToAll
redistributes per-head outputs into per-token-slice outputs (consumed
three cycles later, so the collective is never on the critical path);
each core then does the multiplicative reduce over heads and its slice
of out_proj.

v3: the batch pipeline is software-pipelined one stage deep so the two
hot engines overlap instead of alternating. ACT's exp work (~23us/batch
after pairing+suffix tricks, see below) only exists in the attention
phase, while QKV projection work is almost pure PE; v2 ran them
serially, idling each engine ~half the time. Here cycle bb emits
attention chunks of batch bb-1 INTERLEAVED with the QKV projection of
batch bb (twelve 8-matmul single-bank passes + 16 fp16 PE transposes
distributed across the four q-chunks), so PE fills ACT's exp latency
with projection matmuls.

Other structure (from v2):
- All attention operands fp16 (PE streams 2B dtypes at 2 elem/cycle in
  pipelined matmuls; f32r is 1/cycle and 1/4-rate below N=256; DVE gets
  2x).
- Diagonal-crossing score tiles compute/exp only the valid column
  suffix (N = 512-128j); the causal mask multiply covers just the
  128-wide diagonal block and a memset zeroes the stale prefix.
- exp runs once per PAIR of full score tiles via a 2-bank [128,1024]
  PSUM read: ACT's per-instruction overhead is ~293ns, so halving
  instruction count matters at 40 tiles/batch.
- Softmax denominators: e-tiles pre-summed in quads on DVE, one
  ones-matmul per quad (10/batch).
- Tail (gather + head-product chain on Pool) is emitted at qc==1 three
  cycles after its a2a; its out_proj matmuls are emitted at cycle end so
  PE never waits on the Pool product chain.

PSUM budget (8 banks): score-pairs 2x2 + QKV pass ring 2x1 + acco 1 +
denb 1.
"""
import numpy as np

import concourse.bacc as bacc
import concourse.mybir as mybir
import concourse.tile as tile
from concourse.bass_utils import run_bass_kernel_spmd

B, T, D, H = 4, 2048, 1024, 8
HD = 128          # head dim
P = 128           # partitions
NC = 8            # cores
TQ = 512          # qt chunk width
NKD = D // P      # 8 contraction tiles
NTT = T // P      # 16 token tiles per batch
NQC = T // TQ     # 4 qt chunks per batch
TS = T // NC      # 256: per-core token slice of one batch
TOK_SLICE = B * TS  # 1024 tokens per core in the tail

dt = mybir.dt
F32 = dt.float32
F32R = dt.float32r
F16 = dt.float16

_PROGRAM = {}


def build_program(mode=None, reps=1):
    """reps>1 repeats the whole pipeline inside one NEFF (for timing)."""
    global _PROGRAM
    key = f"v4-{reps}"
    if key in _PROGRAM:
        return _PROGRAM[key]
    nc = bacc.Bacc("TRN2", target_bir_lowering=False, debug=False,
                   num_devices=NC)

    xT_d = nc.dram_tensor("xT", [B, D, T], F16, kind="ExternalInput")
    wq_d = nc.dram_tensor("wq", [B, P, NKD * HD], F16, kind="ExternalInput")
    wk_d = nc.dram_tensor("wk", [B, P, NKD * HD], F16, kind="ExternalInput")
    wv_d = nc.dram_tensor("wv", [B, P, NKD * HD], F16, kind="ExternalInput")
    wo_d = nc.dram_tensor("wo", [HD, D], F32R, kind="ExternalInput")
    cm_d = nc.dram_tensor("cm", [4, P, TQ], F16, kind="ExternalInput")
    eye_d = nc.dram_tensor("eye", [P, P], F16, kind="ExternalInput")
    y_d = nc.dram_tensor("y", [TOK_SLICE, D], F32, kind="ExternalOutput")

    rg = [list(range(NC))]
    Exp = mybir.ActivationFunctionType.Exp

    from contextlib import ExitStack
    with tile.TileContext(nc) as tc, ExitStack() as ctx:
        ec = ctx.enter_context
        constp = ec(tc.tile_pool(name="const", bufs=1))
        xp = ec(tc.tile_pool(name="xp", bufs=2))
        wsp = ec(tc.tile_pool(name="wsp", bufs=2))
        qkvp = ec(tc.tile_pool(name="qkv", bufs=2))
        eop = ec(tc.tile_pool(name="eo", bufs=4))
        esp = ec(tc.tile_pool(name="es", bufs=4))
        scp = ec(tc.tile_pool(name="sc", bufs=3))
        outbp = ec(tc.tile_pool(name="outb", bufs=2))
        prodp = ec(tc.tile_pool(name="prod", bufs=2))
        ysbp = ec(tc.tile_pool(name="ysb", bufs=2))
        psS = ec(tc.tile_pool(name="psS", bufs=2, space="PSUM"))
        psQ = ec(tc.tile_pool(name="psQ", bufs=2, space="PSUM"))
        psO = ec(tc.tile_pool(name="psO", bufs=1, space="PSUM"))
        psD = ec(tc.tile_pool(name="psD", bufs=1, space="PSUM"))
        dram = ec(tc.tile_pool(name="dram", bufs=1, space="DRAM"))

        a2a_in = [dram.tile([NC, P, TS], F16, name=f"a2a_in{b}",
                            tag=f"a2a_in{b}") for b in range(B)]
        a2a_out = [dram.tile([NC, P, TS], F16, name=f"a2a_out{b}",
                             tag=f"a2a_out{b}") for b in range(B)]
        hpp = ec(tc.tile_pool(name="hp", bufs=2))

        ones_h = constp.tile([P, P], F16, name="ones_h", tag="ones_h")
        nc.vector.memset(ones_h[:], 1.0)

        eye_h = constp.tile([P, P], F16, name="eye_h", tag="eye_h")
        nc.sync.dma_start(eye_h[:], eye_d.ap())

        cm_sb = constp.tile([P, 4 * TQ], F16, name="cm_sb", tag="cm_sb")

        wor = constp.tile([P, D], F32R, name="wor", tag="wor")

        # ---- load machinery -------------------------------------------
        staged = {}

        def emit_loads(bb, startup=False):
            """DMA batch bb's x chunks + weights; record tiles in staged."""
            b = bb % B
            st = {}
            staged[bb] = st
            x_sb = xp.tile([P, NKD * T], F16, name="x_sb", tag="x_sb")
            st["x"] = x_sb
            ws = {}
            for nm, wd in (("v", wv_d), ("q", wq_d), ("k", wk_d)):
                ws[nm] = wsp.tile([P, NKD * HD], F16, name=f"ws_{nm}",
                                  tag=f"ws_{nm}")
            st["w"] = ws
            if startup:
                # spread batch-0 loads over Pool/SP/ACT in half-chunk DMAs
                # so the V projection starts ~2us in and is never starved
                nc.scalar.dma_start(ws["v"][:], wv_d.ap()[b])
                engs = [nc.gpsimd, nc.sync, nc.scalar]
                i = 0
                HT = T // 2
                QT4 = T // 4
                for kd in range(NKD):
                    if kd == 0:
                        for h in range(4):
                            engs[i % 3].dma_start(
                                x_sb[:, h * QT4:(h + 1) * QT4],
                                xT_d.ap()[b, 0:P, h * QT4:(h + 1) * QT4])
                            i += 1
                        continue
                    for h in range(2):
                        engs[i % 3].dma_start(
                            x_sb[:, kd * T + h * HT: kd * T + (h + 1) * HT],
                            xT_d.ap()[b, kd * P:(kd + 1) * P,
                                      h * HT:(h + 1) * HT])
                        i += 1
                    if kd == 2:
                        nc.sync.dma_start(ws["q"][:], wq_d.ap()[b])
                    elif kd == 4:
                        nc.gpsimd.dma_start(ws["k"][:], wk_d.ap()[b])
                return
            # steady state: all on SP; few big DMAs
            HK = NKD // 2
            nc.sync.dma_start(ws["v"][:], wv_d.ap()[b])
            nc.sync.dma_start(
                x_sb[:, 0:HK * T],
                xT_d.ap()[b, 0:HK * P, :].rearrange("(kd p) t -> p kd t",
                                                    p=P))
            nc.sync.dma_start(ws["q"][:], wq_d.ap()[b])
            nc.sync.dma_start(
                x_sb[:, HK * T:NKD * T],
                xT_d.ap()[b, HK * P:NKD * P, :].rearrange(
                    "(kd p) t -> p kd t", p=P))
            nc.sync.dma_start(ws["k"][:], wk_d.ap()[b])

        # ---- QKV projection passes ------------------------------------
        # One pass = one q-chunk of one projection: 8 accumulating
        # matmuls (kd 0..7) into a single PSUM bank, then evict to fp16.
        qkvs = {}

        def emit_qkv_pass(bb, proj, qc):
            st = staged[bb]
            xr = st["x"]
            w = st["w"][proj]
            dst = qkvs[bb][proj]
            acc = psQ.tile([P, TQ], F32, name="qacc", tag="qacc")
            for kd in range(NKD):
                nc.tensor.matmul(
                    acc[:],
                    w[:, kd * HD:(kd + 1) * HD],
                    xr[:, kd * T + qc * TQ: kd * T + (qc + 1) * TQ],
                    start=(kd == 0), stop=(kd == NKD - 1))
            nc.any.tensor_copy(dst[:, qc * TQ:(qc + 1) * TQ], acc[:])

        def emit_transposes(bb, lo, hi):
            vT = qkvs[bb]["vT"]
            v_sb = qkvs[bb]["vs"]
            for tt in range(lo, hi):
                vtp = psQ.tile([P, P], F16, name="vtp", tag="qacc")
                nc.tensor.transpose(
                    vtp[:], vT[:, tt * P:(tt + 1) * P], eye_h[:])
                nc.any.tensor_copy(
                    v_sb[:, tt * HD:(tt + 1) * HD], vtp[:])

        def emit_qkv_slot(bb, qc):
            """Slot qc of the QKV(bb) work, interleaved after attention
            chunk qc of batch bb-1."""
            if qc == 0:
                for c in range(NQC):
                    emit_qkv_pass(bb, "v", c)
            elif qc == 1:
                for c in range(NQC):
                    emit_qkv_pass(bb, "q", c)
            elif qc == 2:
                for c in range(2):
                    emit_qkv_pass(bb, "k", c)
                emit_transposes(bb, 0, 8)
            else:
                for c in range(2, NQC):
                    emit_qkv_pass(bb, "k", c)
                emit_transposes(bb, 8, NTT)

        def alloc_qkv(bb):
            qkvs[bb] = {
                "vT": qkvp.tile([P, T], F16, name="vT", tag="vT"),
                "vs": qkvp.tile([P, NTT * HD], F16, name="vS", tag="vS"),
                "q": qkvp.tile([P, T], F16, name="qT", tag="qT"),
                "k": qkvp.tile([P, T], F16, name="kT", tag="kT"),
            }
            qkvs[bb]["v"] = qkvs[bb]["vT"]

        # ---- attention ------------------------------------------------
        def emit_attention_chunk(bb, qc, out_b):
            b = bb % B
            qT = qkvs[bb]["q"]
            kT = qkvs[bb]["k"]
            v_sb = qkvs[bb]["vs"]
            nkt = 4 * (qc + 1)
            nfull = 4 * qc
            acco = psO.tile([P, TQ], F32, name="acco", tag="acco")
            denb = psD.tile([P, TQ], F32, name="denb", tag="denb")
            es_pairs = []

            for pr_i in range(nkt // 2):
                kt0 = 2 * pr_i
                sp = psS.tile([P, 2 * TQ], F32, name="sp", tag="accs")
                ep = eop.tile([P, 2 * TQ], F16, name="ep", tag="ep")
                for h in (0, 1):
                    kt = kt0 + h
                    j = kt - nfull
                    off = 0 if j < 0 else j * P  # valid-column suffix
                    nc.tensor.matmul(
                        sp[:, h * TQ + off:(h + 1) * TQ],
                        kT[:, kt * P:(kt + 1) * P],
                        qT[:, qc * TQ + off:(qc + 1) * TQ],
                        start=True, stop=True)
                j0 = kt0 - nfull
                if j0 < 0:
                    # both tiles full: one exp over the 2-bank span
                    nc.scalar.activation(ep[:], sp[:], Exp)
                else:
                    for h in (0, 1):
                        j = j0 + h
                        off = j * P
                        nc.scalar.activation(
                            ep[:, h * TQ + off:(h + 1) * TQ],
                            sp[:, h * TQ + off:(h + 1) * TQ], Exp)
                        # zero the invalid prefix (stale slot memory) and
                        # mask the diagonal-crossing 128-col block
                        if j > 0:
                            nc.vector.memset(ep[:, h * TQ:h * TQ + off],
                                             0.0)
                        nc.vector.tensor_mul(
                            ep[:, h * TQ + off:h * TQ + off + P],
                            ep[:, h * TQ + off:h * TQ + off + P],
                            cm_sb[:, j * TQ + off:j * TQ + off + P])
                for h in (0, 1):
                    kt = kt0 + h
                    nc.tensor.matmul(
                        acco[:],
                        v_sb[:, kt * HD:(kt + 1) * HD],
                        ep[:, h * TQ:(h + 1) * TQ],
                        start=(kt == 0), stop=(kt == nkt - 1))
                es = esp.tile([P, TQ], F16, name="es", tag="es")
                nc.vector.tensor_add(
                    es[:], ep[:, 0:TQ], ep[:, TQ:2 * TQ])
                es_pairs.append(es)
                if len(es_pairs) == 2:
                    esq = esp.tile([P, TQ], F16, name="esq", tag="esq")
                    nc.vector.tensor_add(
                        esq[:], es_pairs[0][:], es_pairs[1][:])
                    es_pairs = []
                    quad = pr_i // 2
                    nc.tensor.matmul(
                        denb[:], ones_h[:], esq[:],
                        start=(quad == 0), stop=(quad == qc))

            recb = scp.tile([P, TQ], F32, name="recb", tag="recb")
            nc.vector.reciprocal_approx_fast(recb[:], denb[:])
            nc.vector.tensor_mul(
                out_b[:, qc * TQ:(qc + 1) * TQ], acco[:], recb[:])
            # ship this qc's two token slices to the collective buffer
            for j in (2 * qc, 2 * qc + 1):
                nc.gpsimd.dma_start(a2a_in[b][j],
                                    out_b[:, j * TS:(j + 1) * TS])

        # ---- tail: gather + product chain (Pool), then out_proj (PE) --
        prods = {}

        def emit_tail_pool(bb, final=False):
            b = bb % B
            hp = hpp.tile([P, NC * TS], F16, name="hp", tag="hp")
            if final:
                half = NC // 2
                nc.gpsimd.dma_start(
                    hp[:, 0:half * TS],
                    a2a_out[b][0:half].rearrange("r p t -> p r t"))
                nc.scalar.dma_start(
                    hp[:, half * TS:],
                    a2a_out[b][half:NC].rearrange("r p t -> p r t"))
            else:
                nc.gpsimd.dma_start(
                    hp[:], a2a_out[b].rearrange("r p t -> p r t"))
            pr = prodp.tile([P, TS], F16, name="pr", tag="pr")
            nc.gpsimd.tensor_mul(pr[:], hp[:, 0:TS], hp[:, TS:2 * TS])
            for r in range(2, NC - 1):
                nc.gpsimd.tensor_mul(
                    pr[:], pr[:], hp[:, r * TS:(r + 1) * TS])
            prod_r = prodp.tile([P, TS], F32R, name="prod_r", tag="prodr")
            nc.gpsimd.tensor_mul(
                prod_r[:], pr[:], hp[:, (NC - 1) * TS:NC * TS])
            prods[bb] = prod_r

        def emit_tail_pe(bb, final=False):
            b = bb % B
            prod_r = prods.pop(bb)
            for tt in range(TS // P):
                ysb = ysbp.tile([P, D], F32, name="ysb", tag="ysb")
                for nn in range(D // TQ):
                    accy = psO.tile([P, TQ], F32, name="accy", tag="acco")
                    nc.tensor.matmul(
                        accy[:],
                        prod_r[:, tt * P:(tt + 1) * P],
                        wor[:, nn * TQ:(nn + 1) * TQ],
                        start=True, stop=True)
                    nc.vector.tensor_copy(
                        ysb[:, nn * TQ:(nn + 1) * TQ], accy[:])
                row = b * TS + tt * P
                yeng = nc.scalar if (final and tt == 1) else nc.sync
                yeng.dma_start(y_d.ap()[row:row + P, :], ysb[:])

        # ---- pipeline --------------------------------------------------
        emit_loads(0, startup=True)
        nc.scalar.dma_start(cm_sb[:], cm_d.ap().rearrange("j p q -> p j q"))
        nc.gpsimd.dma_start(wor[:], wo_d.ap())

        NB = B * reps

        # prologue: QKV(0) with no attention to interleave
        emit_loads(1)
        alloc_qkv(0)
        for qc in range(NQC):
            emit_qkv_slot(0, qc)

        for bb in range(1, NB + 1):
            ab = bb - 1            # attention batch this cycle
            if bb + 1 < NB:
                emit_loads(bb + 1)
            if bb < NB:
                alloc_qkv(bb)
            out_b = outbp.tile([P, T], F16, name="out_b", tag="out_b")
            for qc in range(NQC):
                emit_attention_chunk(ab, qc, out_b)
                if bb < NB:
                    emit_qkv_slot(bb, qc)
                if qc == 1 and bb - 3 >= 0:
                    emit_tail_pool(bb - 3)
            qkvs.pop(ab)
            b = ab % B
            nc.gpsimd.collective_compute(
                "AllToAll", mybir.AluOpType.bypass,
                replica_groups=rg,
                ins=[a2a_in[b].opt()], outs=[a2a_out[b].opt()])
            if bb - 3 >= 0:
                emit_tail_pe(bb - 3)
            if bb == NB:
                emit_tail_pool(NB - 2)
                emit_tail_pe(NB - 2)

        emit_tail_pool(NB - 1, final=True)
        emit_tail_pe(NB - 1, final=True)

    nc.compile()
    _PROGRAM[key] = nc
    return nc


def make_in_maps(x, Wq, Wk, Wv, Wout, q_mask, k_mask, v_mask):
    x = np.ascontiguousarray(np.asarray(x, np.float32))
    xT = np.ascontiguousarray(x.transpose(0, 2, 1).astype(np.float16))
    wo = np.ascontiguousarray(np.asarray(Wout, np.float32).T)  # (HD, D)

    cm = np.zeros((4, P, TQ), np.float16)
    for j in range(4):
        for i in range(P):
            cm[j, i, j * P + i:] = 1.0
    eye = np.eye(P, dtype=np.float16)

    s = np.float32(1.0 / np.sqrt(HD))
    q_mask = np.asarray(q_mask, np.float32)
    k_mask = np.asarray(k_mask, np.float32)
    v_mask = np.asarray(v_mask, np.float32)
    Wq = np.asarray(Wq, np.float32)
    Wk = np.asarray(Wk, np.float32)
    Wv = np.asarray(Wv, np.float32)

    in_maps = []
    for c in range(NC):
        def pack(W, m, scale):
            out = np.empty((B, P, NKD * HD), np.float16)
            Wh = W[c * HD:(c + 1) * HD, :]                  # (HD, D)
            for b in range(B):
                Wp = (Wh * (m[b, c, 0, :, None] * scale)).T  # (D, HD)
                out[b] = Wp.reshape(NKD, P, HD).transpose(1, 0, 2).reshape(
                    P, NKD * HD)
            return out
        in_maps.append({
            "xT": xT,
            "wq": pack(Wq, q_mask, s),
            "wk": pack(Wk, k_mask, np.float32(1.0)),
            "wv": pack(Wv, v_mask, np.float32(1.0)),
            "wo": wo,
            "cm": cm,
            "eye": eye,
        })
    return in_maps


def kernel(x, Wq, Wk, Wv, Wout, q_mask, k_mask, v_mask, mask=None):
    nc = build_program()
    in_maps = make_in_maps(x, Wq, Wk, Wv, Wout, q_mask, k_mask, v_mask)
    res = run_bass_kernel_spmd(nc, in_maps, core_ids=list(range(NC))).results
    # core c's y rows are ordered (b, local-token); its tokens are
    # [c*TS, (c+1)*TS) of every batch
    out = np.empty((B, T, D), np.float32)
    for c in range(NC):
        yc = res[c]["y"].reshape(B, TS, D)
        out[:, c * TS:(c + 1) * TS, :] = yc
    return out


# revision 6
# speedup vs baseline: 1.0882x; 1.0528x over previous
"""Bass/Trainium2 kernel for nn_BayesMultiheadAttention (B=4,T=2048,D=1024,H=8).

Sharding: tensor-parallel over heads. Core c computes head c (QKV proj +
causal attention) for all 4 batches; a per-batch fp16 AllToAll
redistributes per-head outputs into per-token-slice outputs (consumed
three cycles later, so the collective is never on the critical path);
each core then does the multiplicative reduce over heads and its slice
of out_proj.

v3: the batch pipeline is software-pipelined one stage deep so the two
hot engines overlap instead of alternating. ACT's exp work (~23us/batch
after pairing+suffix tricks, see below) only exists in the attention
phase, while QKV projection work is almost pure PE; v2 ran them
serially, idling each engine ~half the time. Here cycle bb emits
attention chunks of batch bb-1 INTERLEAVED with the QKV projection of
batch bb (twelve 8-matmul single-bank passes + 16 fp16 PE transposes
distributed across the four q-chunks), so PE fills ACT's exp latency
with projection matmuls.

Other structure (from v2):
- All attention operands fp16 (PE streams 2B dtypes at 2 elem/cycle in
  pipelined matmuls; f32r is 1/cycle and 1/4-rate below N=256; DVE gets
  2x).
- Diagonal-crossing score tiles compute/exp only the valid column
  suffix (N = 512-128j); the causal mask multiply covers just the
  128-wide diagonal block and a memset zeroes the stale prefix.
- exp runs once per PAIR of full score tiles via a 2-bank [128,1024]
  PSUM read: ACT's per-instruction overhead is ~293ns, so halving
  instruction count matters at 40 tiles/batch.
- Softmax denominators: e-tiles pre-summed in quads on DVE, one
  ones-matmul per quad (10/batch).
- Tail (gather + head-product chain on Pool) is emitted at qc==1 three
  cycles after its a2a; its out_proj matmuls are emitted at cycle end so
  PE never waits on the Pool product chain.

PSUM budget (8 banks): score-pairs 2x2 + QKV pass ring 2x1 + acco 1 +
denb 1.
"""
import numpy as np

import concourse.bacc as bacc
import concourse.mybir as mybir
import concourse.tile as tile
from concourse.bass_utils import run_bass_kernel_spmd

B, T, D, H = 4, 2048, 1024, 8
HD = 128          # head dim
P = 128           # partitions
NC = 8            # cores
TQ = 512          # qt chunk width
NKD = D // P      # 8 contraction tiles
NTT = T // P      # 16 token tiles per batch
NQC = T // TQ     # 4 qt chunks per batch
TS = T // NC      # 256: per-core token slice of one batch
TOK_SLICE = B * TS  # 1024 tokens per core in the tail

dt = mybir.dt
F32 = dt.float32
F32R = dt.float32r
F16 = dt.float16

_PROGRAM = {}


def build_program(mode=None, reps=1):
    """reps>1 repeats the whole pipeline inside one NEFF (for timing)."""
    global _PROGRAM
    key = f"v4-{reps}"
    if key in _PROGRAM:
        return _PROGRAM[key]
    nc = bacc.Bacc("TRN2", target_bir_lowering=False, debug=False,
                   num_devices=NC)

    xT_d = nc.dram_tensor("xT", [B, D, T], F16, kind="ExternalInput")
    wq_d = nc.dram_tensor("wq", [B, P, NKD * HD], F16, kind="ExternalInput")
    wk_d = nc.dram_tensor("wk", [B, P, NKD * HD], F16, kind="ExternalInput")
    wv_d = nc.dram_tensor("wv", [B, P, NKD * HD], F16, kind="ExternalInput")
    wo_d = nc.dram_tensor("wo", [HD, D], F32R, kind="ExternalInput")
    cm_d = nc.dram_tensor("cm", [4, P, TQ], F16, kind="ExternalInput")
    eye_d = nc.dram_tensor("eye", [P, P], F16, kind="ExternalInput")
    y_d = nc.dram_tensor("y", [TOK_SLICE, D], F32, kind="ExternalOutput")

    rg = [list(range(NC))]
    Exp = mybir.ActivationFunctionType.Exp

    from contextlib import ExitStack
    with tile.TileContext(nc) as tc, ExitStack() as ctx:
        ec = ctx.enter_context
        constp = ec(tc.tile_pool(name="const", bufs=1))
        xp = ec(tc.tile_pool(name="xp", bufs=2))
        wsp = ec(tc.tile_pool(name="wsp", bufs=2))
        qkvp = ec(tc.tile_pool(name="qkv", bufs=2))
        eop = ec(tc.tile_pool(name="eo", bufs=4))
        esp = ec(tc.tile_pool(name="es", bufs=4))
        scp = ec(tc.tile_pool(name="sc", bufs=3))
        outbp = ec(tc.tile_pool(name="outb", bufs=2))
        prodp = ec(tc.tile_pool(name="prod", bufs=2))
        ysbp = ec(tc.tile_pool(name="ysb", bufs=2))
        psS = ec(tc.tile_pool(name="psS", bufs=2, space="PSUM"))
        psQ = ec(tc.tile_pool(name="psQ", bufs=2, space="PSUM"))
        psO = ec(tc.tile_pool(name="psO", bufs=1, space="PSUM"))
        psD = ec(tc.tile_pool(name="psD", bufs=1, space="PSUM"))
        dram = ec(tc.tile_pool(name="dram", bufs=1, space="DRAM"))

        a2a_in = [dram.tile([NC, P, TS], F16, name=f"a2a_in{b}",
                            tag=f"a2a_in{b}") for b in range(B)]
        a2a_out = [dram.tile([NC, P, TS], F16, name=f"a2a_out{b}",
                             tag=f"a2a_out{b}") for b in range(B)]
        hpp = ec(tc.tile_pool(name="hp", bufs=2))

        ones_h = constp.tile([P, P], F16, name="ones_h", tag="ones_h")
        nc.vector.memset(ones_h[:], 1.0)

        eye_h = constp.tile([P, P], F16, name="eye_h", tag="eye_h")
        nc.sync.dma_start(eye_h[:], eye_d.ap())

        cm_sb = constp.tile([P, 4 * TQ], F16, name="cm_sb", tag="cm_sb")

        wor = constp.tile([P, D], F32R, name="wor", tag="wor")

        # ---- load machinery -------------------------------------------
        staged = {}

        def emit_loads(bb, startup=False):
            """DMA batch bb's x chunks + weights; record tiles in staged."""
            b = bb % B
            st = {}
            staged[bb] = st
            x_sb = xp.tile([P, NKD * T], F16, name="x_sb", tag="x_sb")
            st["x"] = x_sb
            ws = {}
            for nm, wd in (("v", wv_d), ("q", wq_d), ("k", wk_d)):
                ws[nm] = wsp.tile([P, NKD * HD], F16, name=f"ws_{nm}",
                                  tag=f"ws_{nm}")
            st["w"] = ws
            if startup:
                # spread batch-0 loads over Pool/SP/ACT in half-chunk DMAs
                # so the V projection starts ~2us in and is never starved
                nc.scalar.dma_start(ws["v"][:], wv_d.ap()[b])
                engs = [nc.gpsimd, nc.sync, nc.scalar]
                i = 0
                HT = T // 2
                QT4 = T // 4
                for kd in range(NKD):
                    if kd == 0:
                        for h in range(4):
                            engs[i % 3].dma_start(
                                x_sb[:, h * QT4:(h + 1) * QT4],
                                xT_d.ap()[b, 0:P, h * QT4:(h + 1) * QT4])
                            i += 1
                        continue
                    for h in range(2):
                        engs[i % 3].dma_start(
                            x_sb[:, kd * T + h * HT: kd * T + (h + 1) * HT],
                            xT_d.ap()[b, kd * P:(kd + 1) * P,
                                      h * HT:(h + 1) * HT])
                        i += 1
                    if kd == 2:
                        nc.sync.dma_start(ws["q"][:], wq_d.ap()[b])
                    elif kd == 4:
                        nc.gpsimd.dma_start(ws["k"][:], wk_d.ap()[b])
                return
            # steady state: all on SP; few big DMAs
            HK = NKD // 2
            nc.sync.dma_start(ws["v"][:], wv_d.ap()[b])
            nc.sync.dma_start(
                x_sb[:, 0:HK * T],
                xT_d.ap()[b, 0:HK * P, :].rearrange("(kd p) t -> p kd t",
                                                    p=P))
            nc.sync.dma_start(ws["q"][:], wq_d.ap()[b])
            nc.sync.dma_start(
                x_sb[:, HK * T:NKD * T],
                xT_d.ap()[b, HK * P:NKD * P, :].rearrange(
                    "(kd p) t -> p kd t", p=P))
            nc.sync.dma_start(ws["k"][:], wk_d.ap()[b])

        # ---- QKV projection passes ------------------------------------
        # One pass = one q-chunk of one projection: 8 accumulating
        # matmuls (kd 0..7) into a single PSUM bank, then evict to fp16.
        qkvs = {}

        def emit_qkv_pass(bb, proj, qc):
            st = staged[bb]
            xr = st["x"]
            w = st["w"][proj]
            dst = qkvs[bb][proj]
            acc = psQ.tile([P, TQ], F32, name="qacc", tag="qacc")
            for kd in range(NKD):
                nc.tensor.matmul(
                    acc[:],
                    w[:, kd * HD:(kd + 1) * HD],
                    xr[:, kd * T + qc * TQ: kd * T + (qc + 1) * TQ],
                    start=(kd == 0), stop=(kd == NKD - 1))
            nc.any.tensor_copy(dst[:, qc * TQ:(qc + 1) * TQ], acc[:])

        def emit_transposes(bb, lo, hi):
            vT = qkvs[bb]["vT"]
            v_sb = qkvs[bb]["vs"]
            for tt in range(lo, hi):
                vtp = psQ.tile([P, P], F16, name="vtp", tag="qacc")
                nc.tensor.transpose(
                    vtp[:], vT[:, tt * P:(tt + 1) * P], eye_h[:])
                nc.any.tensor_copy(
                    v_sb[:, tt * HD:(tt + 1) * HD], vtp[:])

        def emit_qkv_slot(bb, qc):
            """Slot qc of the QKV(bb) work, interleaved after attention
            chunk qc of batch bb-1."""
            if qc == 0:
                for c in range(NQC):
                    emit_qkv_pass(bb, "v", c)
            elif qc == 1:
                for c in range(NQC):
                    emit_qkv_pass(bb, "q", c)
            elif qc == 2:
                for c in range(2):
                    emit_qkv_pass(bb, "k", c)
                emit_transposes(bb, 0, 8)
            else:
                for c in range(2, NQC):
                    emit_qkv_pass(bb, "k", c)
                emit_transposes(bb, 8, NTT)

        def alloc_qkv(bb):
            qkvs[bb] = {
                "vT": qkvp.tile([P, T], F16, name="vT", tag="vT"),
                "vs": qkvp.tile([P, NTT * HD], F16, name="vS", tag="vS"),
                "q": qkvp.tile([P, T], F16, name="qT", tag="qT"),
                "k": qkvp.tile([P, T], F16, name="kT", tag="kT"),
            }
            qkvs[bb]["v"] = qkvs[bb]["vT"]

        # ---- attention ------------------------------------------------
        def emit_attention_chunk(bb, qc, out_b):
            b = bb % B
            qT = qkvs[bb]["q"]
            kT = qkvs[bb]["k"]
            v_sb = qkvs[bb]["vs"]
            nkt = 4 * (qc + 1)
            nfull = 4 * qc
            acco = psO.tile([P, TQ], F32, name="acco", tag="acco")
            denb = psD.tile([P, TQ], F32, name="denb", tag="denb")
            es_pairs = []

            for pr_i in range(nkt // 2):
                kt0 = 2 * pr_i
                sp = psS.tile([P, 2 * TQ], F32, name="sp", tag="accs")
                ep = eop.tile([P, 2 * TQ], F16, name="ep", tag="ep")
                for h in (0, 1):
                    kt = kt0 + h
                    j = kt - nfull
                    off = 0 if j < 0 else j * P  # valid-column suffix
                    nc.tensor.matmul(
                        sp[:, h * TQ + off:(h + 1) * TQ],
                        kT[:, kt * P:(kt + 1) * P],
                        qT[:, qc * TQ + off:(qc + 1) * TQ],
                        start=True, stop=True)
                j0 = kt0 - nfull
                if j0 < 0:
                    # both tiles full: one exp over the 2-bank span
                    nc.scalar.activation(ep[:], sp[:], Exp)
                else:
                    for h in (0, 1):
                        j = j0 + h
                        off = j * P
                        nc.scalar.activation(
                            ep[:, h * TQ + off:(h + 1) * TQ],
                            sp[:, h * TQ + off:(h + 1) * TQ], Exp)
                        # zero the invalid prefix (stale slot memory) and
                        # mask the diagonal-crossing 128-col block
                        if j > 0:
                            nc.vector.memset(ep[:, h * TQ:h * TQ + off],
                                             0.0)
                        nc.vector.tensor_mul(
                            ep[:, h * TQ + off:h * TQ + off + P],
                            ep[:, h * TQ + off:h * TQ + off + P],
                            cm_sb[:, j * TQ + off:j * TQ + off + P])
                for h in (0, 1):
                    kt = kt0 + h
                    j = kt - nfull
                    # diag tiles: the e prefix is zero, so only the valid
                    # column suffix contributes; accumulate the subrange
                    # (has_written bits were set by the kt==0 full-width
                    # matmul, or by j==0 full width when qc==0)
                    off = 0 if j <= 0 else j * P
                    nc.tensor.matmul(
                        acco[:, off:TQ],
                        v_sb[:, kt * HD:(kt + 1) * HD],
                        ep[:, h * TQ + off:(h + 1) * TQ],
                        start=(kt == 0), stop=(kt == nkt - 1),
                        skip_group_check=(off > 0 or kt == nkt - 1))
                es = esp.tile([P, TQ], F16, name="es", tag="es")
                nc.vector.tensor_add(
                    es[:], ep[:, 0:TQ], ep[:, TQ:2 * TQ])
                es_pairs.append(es)
                if len(es_pairs) == 2:
                    esq = esp.tile([P, TQ], F16, name="esq", tag="esq")
                    nc.vector.tensor_add(
                        esq[:], es_pairs[0][:], es_pairs[1][:])
                    es_pairs = []
                    quad = pr_i // 2
                    nc.tensor.matmul(
                        denb[:], ones_h[:], esq[:],
                        start=(quad == 0), stop=(quad == qc))

            recb = scp.tile([P, TQ], F32, name="recb", tag="recb")
            nc.vector.reciprocal_approx_fast(recb[:], denb[:])
            nc.vector.tensor_mul(
                out_b[:, qc * TQ:(qc + 1) * TQ], acco[:], recb[:])
            # ship this qc's two token slices to the collective buffer
            for j in (2 * qc, 2 * qc + 1):
                nc.gpsimd.dma_start(a2a_in[b][j],
                                    out_b[:, j * TS:(j + 1) * TS])

        # ---- tail: gather + product chain (Pool), then out_proj (PE) --
        prods = {}

        def emit_tail_pool(bb, final=False):
            b = bb % B
            hp = hpp.tile([P, NC * TS], F16, name="hp", tag="hp")
            if final:
                half = NC // 2
                nc.gpsimd.dma_start(
                    hp[:, 0:half * TS],
                    a2a_out[b][0:half].rearrange("r p t -> p r t"))
                nc.scalar.dma_start(
                    hp[:, half * TS:],
                    a2a_out[b][half:NC].rearrange("r p t -> p r t"))
            else:
                nc.gpsimd.dma_start(
                    hp[:], a2a_out[b].rearrange("r p t -> p r t"))
            pr = prodp.tile([P, TS], F16, name="pr", tag="pr")
            nc.gpsimd.tensor_mul(pr[:], hp[:, 0:TS], hp[:, TS:2 * TS])
            for r in range(2, NC - 1):
                nc.gpsimd.tensor_mul(
                    pr[:], pr[:], hp[:, r * TS:(r + 1) * TS])
            prod_r = prodp.tile([P, TS], F32R, name="prod_r", tag="prodr")
            nc.gpsimd.tensor_mul(
                prod_r[:], pr[:], hp[:, (NC - 1) * TS:NC * TS])
            prods[bb] = prod_r

        def emit_tail_pe(bb, final=False):
            b = bb % B
            prod_r = prods.pop(bb)
            for tt in range(TS // P):
                ysb = ysbp.tile([P, D], F32, name="ysb", tag="ysb")
                for nn in range(D // TQ):
                    accy = psO.tile([P, TQ], F32, name="accy", tag="acco")
                    nc.tensor.matmul(
                        accy[:],
                        prod_r[:, tt * P:(tt + 1) * P],
                        wor[:, nn * TQ:(nn + 1) * TQ],
                        start=True, stop=True)
                    nc.vector.tensor_copy(
                        ysb[:, nn * TQ:(nn + 1) * TQ], accy[:])
                row = b * TS + tt * P
                yeng = nc.scalar if (final and tt == 1) else nc.sync
                yeng.dma_start(y_d.ap()[row:row + P, :], ysb[:])

        # ---- pipeline --------------------------------------------------
        emit_loads(0, startup=True)
        nc.scalar.dma_start(cm_sb[:], cm_d.ap().rearrange("j p q -> p j q"))
        nc.gpsimd.dma_start(wor[:], wo_d.ap())

        NB = B * reps

        # prologue: QKV(0) with no attention to interleave
        emit_loads(1)
        alloc_qkv(0)
        for qc in range(NQC):
            emit_qkv_slot(0, qc)

        for bb in range(1, NB + 1):
            ab = bb - 1            # attention batch this cycle
            if bb + 1 < NB:
                emit_loads(bb + 1)
            if bb < NB:
                alloc_qkv(bb)
            out_b = outbp.tile([P, T], F16, name="out_b", tag="out_b")
            for qc in range(NQC):
                emit_attention_chunk(ab, qc, out_b)
                if bb < NB:
                    emit_qkv_slot(bb, qc)
                if qc == 1 and bb - 3 >= 0:
                    emit_tail_pool(bb - 3)
            qkvs.pop(ab)
            b = ab % B
            nc.gpsimd.collective_compute(
                "AllToAll", mybir.AluOpType.bypass,
                replica_groups=rg,
                ins=[a2a_in[b].opt()], outs=[a2a_out[b].opt()])
            if bb - 3 >= 0:
                emit_tail_pe(bb - 3)
            if bb == NB:
                emit_tail_pool(NB - 2)
                emit_tail_pe(NB - 2)

        emit_tail_pool(NB - 1, final=True)
        emit_tail_pe(NB - 1, final=True)

    nc.compile()
    _PROGRAM[key] = nc
    return nc


def make_in_maps(x, Wq, Wk, Wv, Wout, q_mask, k_mask, v_mask):
    x = np.ascontiguousarray(np.asarray(x, np.float32))
    xT = np.ascontiguousarray(x.transpose(0, 2, 1).astype(np.float16))
    wo = np.ascontiguousarray(np.asarray(Wout, np.float32).T)  # (HD, D)

    cm = np.zeros((4, P, TQ), np.float16)
    for j in range(4):
        for i in range(P):
            cm[j, i, j * P + i:] = 1.0
    eye = np.eye(P, dtype=np.float16)

    s = np.float32(1.0 / np.sqrt(HD))
    q_mask = np.asarray(q_mask, np.float32)
    k_mask = np.asarray(k_mask, np.float32)
    v_mask = np.asarray(v_mask, np.float32)
    Wq = np.asarray(Wq, np.float32)
    Wk = np.asarray(Wk, np.float32)
    Wv = np.asarray(Wv, np.float32)

    in_maps = []
    for c in range(NC):
        def pack(W, m, scale):
            out = np.empty((B, P, NKD * HD), np.float16)
            Wh = W[c * HD:(c + 1) * HD, :]                  # (HD, D)
            for b in range(B):
                Wp = (Wh * (m[b, c, 0, :, None] * scale)).T  # (D, HD)
                out[b] = Wp.reshape(NKD, P, HD).transpose(1, 0, 2).reshape(
                    P, NKD * HD)
            return out
        in_maps.append({
            "xT": xT,
            "wq": pack(Wq, q_mask, s),
            "wk": pack(Wk, k_mask, np.float32(1.0)),
            "wv": pack(Wv, v_mask, np.float32(1.0)),
            "wo": wo,
            "cm": cm,
            "eye": eye,
        })
    return in_maps


def kernel(x, Wq, Wk, Wv, Wout, q_mask, k_mask, v_mask, mask=None):
    nc = build_program()
    in_maps = make_in_maps(x, Wq, Wk, Wv, Wout, q_mask, k_mask, v_mask)
    res = run_bass_kernel_spmd(nc, in_maps, core_ids=list(range(NC))).results
    # core c's y rows are ordered (b, local-token); its tokens are
    # [c*TS, (c+1)*TS) of every batch
    out = np.empty((B, T, D), np.float32)
    for c in range(NC):
        yc = res[c]["y"].reshape(B, TS, D)
        out[:, c * TS:(c + 1) * TS, :] = yc
    return out
